# revision 13
# baseline (speedup 1.0000x reference)
"""Trainium2 Bass kernel for nn_Difference_RNN: x_t = W_A x_{t-1} + b_A + W_B u_t + b_B,
output = all T states [T, D].  D=4096, C=512, T=2048, 8 NeuronCores.

Algorithm (k=3 blocked halo scan, tensor-parallel, transposed matmuls):
  * Halo-chunked batched scan as before (L=16 chunks, H=11 halo, J=128 lanes
    advancing in lockstep from the zero state; lane 0 gets x_0 injected), but
    the state now advances THREE steps per exchange round:
        x_{p+3} = A^3 x_p + A^2 v_{p+1} + A v_{p+2} + v_{p+3}
    with (A^2)_r, (A^3)_r row-slices precomputed on the host.  Only x_{p+3}
    is AllGathered; the intermediate outputs x_{p+1}, x_{p+2} are computed
    as row-slices directly from the gathered x_p.  9 rounds -> 8 AllGathers
    instead of 50 (the baseline's serialized ~10us-each AG stream was 640us
    of the 722us wall).
  * The v-correction terms never materialize full v vectors: since
    v = W_B u + b, each correction is (A^d W_B)_r u_t + const, and the
    512x512 matrices M_d = (A^d W_B)_r are host-precomputed, so corrections
    are cheap contraction-512 GEMMs off the raw u (stored once in SBUF with
    a 16-column zero pad that realizes the halo automatically).  The bias
    consts fold into the PSUM->SBUF epilogue copies; lane-0 halo/x_0 edge
    cases are two host-shipped 512-vectors applied to partition 0.
  * Matmuls run transposed: the 128-lane state tile is the PE stationary
    operand and the A-power slices stream as 512-wide moving operands, so
    the PE runs at full rate (the baseline's per-step ldweights of every
    A tile was ~45% of PE time).  The psum state [lanes, rows] is PE-
    transposed (4 identity matmuls) back to [rows, lanes] before the
    AllGather so the gathered buffer feeds the next round's stationaries
    directly.
  * fp32 outputs leave as raw [16, lane, row] slices; the host reassembles
    [T, D].  Collective bounce buffers are unique per round; PSUM uses all
    8 banks (state/transpose/out1/out2, double-buffered).
"""

from dataclasses import dataclass

import numpy as np
import ml_dtypes

import concourse.bass as bass
import concourse.bacc as bacc
import concourse.tile as tile
import concourse.mybir as mybir
from concourse import bass_utils
from concourse.masks import make_identity

BF16 = ml_dtypes.bfloat16
F32 = mybir.dt.float32
BF = mybir.dt.bfloat16


@dataclass(frozen=True)
class Cfg:
    D: int = 4096
    C: int = 512
    T: int = 2048
    L: int = 16
    H: int = 11
    KB: int = 3          # steps advanced per exchange round
    n_cores: int = 8

    @property
    def J(self):
        return self.T // self.L          # 128 lanes

    @property
    def R(self):
        return (self.L + self.H) // self.KB  # 9 rounds

    @property
    def RPC(self):
        return self.D // self.n_cores    # 512 rows per core

    @property
    def KT(self):
        return self.D // 128             # 32 contraction chunks

    @property
    def Q(self):
        return self.RPC // 128           # 4 row chunks per core

    @property
    def CTU(self):
        return self.C // 128             # 4 u-contraction chunks

    @property
    def TP(self):
        return self.T + 16               # u pad length (16 leading zeros)


CFG = Cfg()


def build(tc: "tile.TileContext", cfg: Cfg = CFG) -> None:
    nc = tc.nc
    D, C, T, L, H, KB = cfg.D, cfg.C, cfg.T, cfg.L, cfg.H, cfg.KB
    J, R, RPC, KT, Q, CTU, TP = (
        cfg.J, cfg.R, cfg.RPC, cfg.KT, cfg.Q, cfg.CTU, cfg.TP,
    )
    N_CORES = cfg.n_cores
    KHF = KT // 2
    OUT_P = range(H, H + L)              # output positions 11..26

    wa_in = [
        nc.dram_tensor(f"wa{j}_in", [D, RPC], BF, kind="ExternalInput")
        for j in (1, 2, 3)
    ]
    wm_in = [
        nc.dram_tensor(f"wm{d}_in", [C, RPC], BF, kind="ExternalInput")
        for d in (0, 1, 2)
    ]
    u_in = nc.dram_tensor("u_in", [C, T], BF, kind="ExternalInput")
    const_in = nc.dram_tensor("const_in", [3, 128, RPC], F32, kind="ExternalInput")
    fx_in = nc.dram_tensor("fx_in", [2, RPC], F32, kind="ExternalInput")
    out_raw = nc.dram_tensor("out_raw", [L, J, RPC], F32, kind="ExternalOutput")

    with (
        tc.tile_pool(name="const", bufs=1) as const_pool,
        tc.tile_pool(name="state", bufs=2) as state_pool,
        tc.tile_pool(name="shard", bufs=2) as shard_pool,
        tc.tile_pool(name="outst", bufs=4) as out_pool,
        tc.tile_pool(name="psum", bufs=1, space="PSUM") as psum_pool,
        tc.tile_pool(name="dram", bufs=R - 1, space="DRAM") as dram_pool,
    ):
        # ---- resident weights / activations -------------------------------
        # u first (needed by round-0 c-GEMM), then A powers in first-use order.
        upad = const_pool.tile([128, CTU * TP], BF)
        u4 = upad[:].rearrange("p (ct t) -> p ct t", ct=CTU)
        for ct in range(CTU):
            nc.vector.memset(u4[:, ct, 0:16], 0.0)
        nc.scalar.dma_start(
            u4[:, :, 16:TP],
            u_in[:].rearrange("(ct p) t -> p ct t", p=128),
        )
        wm = []
        for d in range(3):
            t_ = const_pool.tile([128, CTU * RPC], BF, tag=f"wm{d}", name=f"wm{d}")
            nc.scalar.dma_start(
                t_[:].rearrange("p (ct m) -> p ct m", ct=CTU),
                wm_in[d][:].rearrange("(ct p) m -> p ct m", p=128),
            )
            wm.append(t_)
        cst = const_pool.tile([128, 3 * RPC], F32)
        nc.scalar.dma_start(
            cst[:].rearrange("p (j m) -> p j m", j=3),
            const_in[:].rearrange("j p m -> p j m"),
        )
        fxs = const_pool.tile([1, RPC], F32, tag="fxs", name="fxs")
        nc.scalar.dma_start(fxs[:], fx_in[0:1, :])
        fxo = const_pool.tile([1, RPC], F32, tag="fxo", name="fxo")
        nc.scalar.dma_start(fxo[:], fx_in[1:2, :])
        # all v-correction vectors (consts + lane-0 fixups folded) are
        # precomputed into cbuf by prologue GEMMs that fill the PE idle
        # time of the early (output-free) rounds
        NSLOT = (R - 1) + sum(
            1 for r_ in range(R) for i_ in range(1, KB)
            if KB * r_ + i_ in OUT_P
        )
        cbuf = const_pool.tile([128, NSLOT * RPC], F32, tag="cbuf", name="cbuf")
        ident = const_pool.tile([128, 128], BF)
        make_identity(nc, ident[:])
        wa = {}
        for j in (3, 1, 2):
            t_ = const_pool.tile([128, KT * RPC], BF, tag=f"wa{j}", name=f"wa{j}")
            nc.scalar.dma_start(
                t_[:].rearrange("p (kt m) -> p kt m", kt=KT),
                wa_in[j - 1][:].rearrange("(kt p) m -> p kt m", p=128),
            )
            wa[j] = t_

        u5 = upad[:].rearrange("p (ct j l) -> p ct j l", ct=CTU, l=16)
        JB = TP // 16
        RSTAR = (H - 2) // KB            # round whose v-window covers x_0

        HRP = RPC // 2

        def fslice(h):
            # f-range of output rows: h=None -> full, else 256-row half
            return slice(0, RPC) if h is None else slice(h * HRP, (h + 1) * HRP)

        def c_gemm(ps, r, i):
            """Accumulate the v-correction GEMMs for position p=KB*r+i into ps."""
            n = i * CTU
            m = 0
            for ii in range(1, i + 1):
                d = i - ii
                base = KB * r + ii + 16 - H
                j0, l = divmod(base, 16)
                assert 0 <= j0 and j0 + J <= JB
                for ct in range(CTU):
                    m += 1
                    nc.tensor.matmul(
                        ps[:],
                        u5[:, ct, j0 : j0 + J, l],
                        wm[d][:, ct * RPC : (ct + 1) * RPC],
                        start=(m == 1),
                        stop=(m == n),
                    )

        cslot = {}
        pc_tags = ["ps10", "ps11", "ps20", "ps21"]
        mains = [(r_, KB) for r_ in range(R - 1)] + [
            (r_, i_) for r_ in range(R) for i_ in range(1, KB)
            if KB * r_ + i_ in OUT_P
        ]
        for ns, (r_, i_) in enumerate(mains):
            pc = psum_pool.tile(
                [128, RPC], F32, tag=pc_tags[ns % 4], name=f"pc_{r_}_{i_}"
            )
            c_gemm(pc, r_, i_)
            cs = cbuf[:, ns * RPC : (ns + 1) * RPC]
            nc.vector.tensor_add(cs, pc[:], cst[:, (i_ - 1) * RPC : i_ * RPC])
            p_ = KB * r_ + i_
            if p_ < H - 1:
                # lane-0 halo: no bias (state is exactly zero there)
                nc.vector.tensor_copy(cs[0:1, :], pc[0:1, :])
            elif r_ == RSTAR and i_ == KB:
                nc.vector.tensor_add(cs[0:1, :], pc[0:1, :], fxs[:])
            elif r_ == RSTAR and i_ == 2:
                nc.vector.tensor_add(cs[0:1, :], pc[0:1, :], fxo[:])
            cslot[(r_, i_)] = ns

        def a_parts(xt):
            # contraction chunks ordered so the first AllGather half's rows
            # (kt = 0,1 mod 4) are consumed first
            for src in (0, 1):
                for m in range(KHF):
                    c, q2 = divmod(m, 2)
                    yield xt[src][:, m * J : (m + 1) * J], c * 4 + src * 2 + q2

        def a_main(ps, xt, i, h=None):
            """ps = (A^i x)^T (stationary = state, moving = A^i)."""
            fs = fslice(h)
            n = 0
            for stat, kt in a_parts(xt):
                n += 1
                nc.tensor.matmul(
                    ps[:, fs],
                    stat,
                    wa[i][:, kt * RPC + fs.start : kt * RPC + fs.stop],
                    start=(n == 1),
                    stop=(n == KT),
                )

        def epilogue(ps, r, i, dst, h=None):
            """dst = ps + c (precomputed correction incl. consts/fixups)."""
            fs = fslice(h)
            ns = cslot[(r, i)]
            cs = slice(ns * RPC + fs.start, ns * RPC + fs.stop)
            if ps is None:
                nc.vector.tensor_copy(dst[:, fs], cbuf[:, cs])
            else:
                nc.vector.tensor_add(dst[:, fs], ps[:, fs], cbuf[:, cs])

        x_tiles = None
        for r in range(R):
            p3 = KB * r + KB
            # ---- state main: x_{p3} = A^3 x + c3, split into f-halves ------
            # each half transposes, exchanges, and redistributes on its own,
            # so the two 512KB AllGathers pipeline within the round
            if r < R - 1:
                ps3 = psum_pool.tile([128, RPC], F32, tag=f"ps3{r % 2}", name=f"ps3_r{r}")
                sh = shard_pool.tile([128, RPC], BF, tag="sh", name=f"sh_r{r}")
                pt = psum_pool.tile([128, RPC], BF, tag=f"pt{r % 2}", name=f"pt_r{r}")
                xsh = shard_pool.tile([128, RPC], BF, tag="xsh", name=f"xsh_r{r}")
                out_bs = []
                for h in (0, 1):
                    if r > 0:
                        a_main(ps3, x_tiles, KB, h=h)
                        epilogue(ps3, r, KB, sh, h=h)
                    else:
                        epilogue(None, r, KB, sh, h=h)
                    for q in (2 * h, 2 * h + 1):
                        nc.tensor.transpose(
                            pt[:, q * 128 : (q + 1) * 128],
                            sh[:, q * 128 : (q + 1) * 128],
                            ident[:],
                        )
                    hs = fslice(h)
                    nc.vector.tensor_copy(xsh[:, hs], pt[:, hs])
                    in_b = dram_pool.tile([HRP, J], BF, tag=f"inb{h}", name=f"inb{h}_r{r}")
                    out_b = dram_pool.tile(
                        [N_CORES * HRP, J], BF, tag=f"outb{h}",
                        addr_space="Shared", name=f"outb{h}_r{r}",
                    )
                    nc.sync.dma_start(
                        in_b[:].rearrange("(q p) l -> p q l", p=128),
                        xsh[:, hs].rearrange("p (q l) -> p q l", q=2),
                    )
                    nc.gpsimd.collective_compute(
                        "AllGather",
                        mybir.AluOpType.bypass,
                        replica_groups=[list(range(N_CORES))],
                        ins=[in_b.opt()],
                        outs=[out_b.opt()],
                    )
                    out_bs.append(out_b)
                nxt = r + 1
                xn = []
                for h, tg in ((0, "xa"), (1, "xb")):
                    xt_ = state_pool.tile([128, KHF * J], BF, tag=tg, name=f"{tg}_r{nxt}")
                    nc.sync.dma_start(
                        xt_[:].rearrange("p (cq l) -> p cq l", cq=KHF),
                        out_bs[h][:].rearrange("(cq p) l -> p cq l", p=128),
                    )
                    xn.append(xt_)
                # state doubles as output at positions p3 in [H, H+L)
                if p3 in OUT_P:
                    ot = out_pool.tile([128, RPC], F32, tag="ot", name=f"ot3_r{r}")
                    epilogue(ps3, r, KB, ot)
                    nc.scalar.dma_start(out_raw[p3 - H], ot[:])
            # ---- intermediate outputs: x_{p} = A^i x + c_i, i < KB --------
            for i in range(1, KB):
                p = KB * r + i
                if p not in OUT_P:
                    continue
                psi = psum_pool.tile([128, RPC], F32, tag=f"ps{i}{r % 2}", name=f"ps{i}_r{r}")
                a_main(psi, x_tiles, i)
                ot = out_pool.tile([128, RPC], F32, tag="ot", name=f"ot{i}_r{r}")
                epilogue(psi, r, i, ot)
                nc.scalar.dma_start(out_raw[p - H], ot[:])
            if r < R - 1:
                x_tiles = (xn[0], xn[1])


def make_program(cfg: Cfg = CFG):
    nc = bacc.Bacc(
        "TRN2", target_bir_lowering=False, debug=False, num_devices=cfg.n_cores
    )
    with tile.TileContext(nc) as tc:
        build(tc, cfg)
    nc.compile()
    return nc


def make_in_maps(x_0, u, W_A, b_A, W_B, b_B, cfg: Cfg = CFG):
    A = np.asarray(W_A, np.float32)
    W_B = np.asarray(W_B, np.float32)
    bt = (np.asarray(b_A) + np.asarray(b_B)).astype(np.float32)
    x_0 = np.asarray(x_0, np.float32)
    A2 = A @ A
    A3 = A2 @ A
    Abt = A @ bt
    A2bt = A2 @ bt
    const1 = bt
    const2 = bt + Abt
    const3 = bt + Abt + A2bt
    xd = x_0 - bt
    fxs_full = const3 + A2 @ xd          # replaces const3 on lane 0 at r=3, i=3
    fxo_full = const2 + A @ xd           # replaces const2 on lane 0 at r=3, i=2
    u_b = np.ascontiguousarray(np.asarray(u).astype(BF16))
    in_maps = []
    for r in range(cfg.n_cores):
        rows = slice(r * cfg.RPC, (r + 1) * cfg.RPC)
        consts = np.stack([
            np.tile(const1[rows], (128, 1)),
            np.tile(const2[rows], (128, 1)),
            np.tile(const3[rows], (128, 1)),
        ]).astype(np.float32)
        fx = np.stack([fxs_full[rows], fxo_full[rows]]).astype(np.float32)
        m = {
            "u_in": u_b,
            "const_in": np.ascontiguousarray(consts),
            "fx_in": np.ascontiguousarray(fx),
        }
        for j, Aj in ((1, A), (2, A2), (3, A3)):
            m[f"wa{j}_in"] = np.ascontiguousarray(Aj[rows, :].T.astype(BF16))
        for d, Ad in ((0, None), (1, A), (2, A2)):
            Mr = W_B[rows, :] if Ad is None else Ad[rows, :] @ W_B
            m[f"wm{d}_in"] = np.ascontiguousarray(Mr.T.astype(BF16))
        in_maps.append(m)
    return in_maps


def assemble_output(results, cfg: Cfg = CFG):
    out = np.empty((cfg.T, cfg.D), np.float32)
    for r in range(cfg.n_cores):
        raw = np.asarray(results[r]["out_raw"])      # [L, J, RPC]
        out[:, r * cfg.RPC : (r + 1) * cfg.RPC] = (
            raw.transpose(1, 0, 2).reshape(cfg.T, cfg.RPC)
        )
    return out


_CACHE: dict = {}


def kernel(**inputs):
    if "nc" not in _CACHE:
        _CACHE["nc"] = make_program()
    nc = _CACHE["nc"]
    in_maps = make_in_maps(
        inputs["x_0"], inputs["u"], inputs["W_A"],
        inputs["b_A"], inputs["W_B"], inputs["b_B"],
    )
    res = bass_utils.run_bass_kernel_spmd(
        nc, in_maps, core_ids=list(range(CFG.n_cores))
    )
    return assemble_output(res.results)


# revision 14
# speedup vs baseline: 1.0405x; 1.0405x over previous
"""Trainium2 Bass kernel for nn_Difference_RNN: x_t = W_A x_{t-1} + b_A + W_B u_t + b_B,
output = all T states [T, D].  D=4096, C=512, T=2048, 8 NeuronCores.

Algorithm (k=3 blocked halo scan, tensor-parallel, transposed matmuls):
  * Halo-chunked batched scan as before (L=16 chunks, H=11 halo, J=128 lanes
    advancing in lockstep from the zero state; lane 0 gets x_0 injected), but
    the state now advances THREE steps per exchange round:
        x_{p+3} = A^3 x_p + A^2 v_{p+1} + A v_{p+2} + v_{p+3}
    with (A^2)_r, (A^3)_r row-slices precomputed on the host.  Only x_{p+3}
    is AllGathered; the intermediate outputs x_{p+1}, x_{p+2} are computed
    as row-slices directly from the gathered x_p.  9 rounds -> 8 AllGathers
    instead of 50 (the baseline's serialized ~10us-each AG stream was 640us
    of the 722us wall).
  * The v-correction terms never materialize full v vectors: since
    v = W_B u + b, each correction is (A^d W_B)_r u_t + const, and the
    512x512 matrices M_d = (A^d W_B)_r are host-precomputed, so corrections
    are cheap contraction-512 GEMMs off the raw u (stored once in SBUF with
    a 16-column zero pad that realizes the halo automatically).  The bias
    consts fold into the PSUM->SBUF epilogue copies; lane-0 halo/x_0 edge
    cases are two host-shipped 512-vectors applied to partition 0.
  * Matmuls run transposed: the 128-lane state tile is the PE stationary
    operand and the A-power slices stream as 512-wide moving operands, so
    the PE runs at full rate (the baseline's per-step ldweights of every
    A tile was ~45% of PE time).  The psum state [lanes, rows] is PE-
    transposed (4 identity matmuls) back to [rows, lanes] before the
    AllGather so the gathered buffer feeds the next round's stationaries
    directly.
  * fp32 outputs leave as raw [16, lane, row] slices; the host reassembles
    [T, D].  Collective bounce buffers are unique per round; PSUM uses all
    8 banks (state/transpose/out1/out2, double-buffered).
"""

from dataclasses import dataclass

import numpy as np
import ml_dtypes

import concourse.bass as bass
import concourse.bacc as bacc
import concourse.tile as tile
import concourse.mybir as mybir
from concourse import bass_utils
from concourse.masks import make_identity

BF16 = ml_dtypes.bfloat16
F32 = mybir.dt.float32
BF = mybir.dt.bfloat16


@dataclass(frozen=True)
class Cfg:
    D: int = 4096
    C: int = 512
    T: int = 2048
    L: int = 16
    H: int = 11
    KB: int = 3          # steps advanced per exchange round
    n_cores: int = 8

    @property
    def J(self):
        return self.T // self.L          # 128 lanes

    @property
    def R(self):
        return (self.L + self.H) // self.KB  # 9 rounds

    @property
    def RPC(self):
        return self.D // self.n_cores    # 512 rows per core

    @property
    def KT(self):
        return self.D // 128             # 32 contraction chunks

    @property
    def Q(self):
        return self.RPC // 128           # 4 row chunks per core

    @property
    def CTU(self):
        return self.C // 128             # 4 u-contraction chunks

    @property
    def TP(self):
        return self.T + 16               # u pad length (16 leading zeros)


CFG = Cfg()


def build(tc: "tile.TileContext", cfg: Cfg = CFG) -> None:
    nc = tc.nc
    D, C, T, L, H, KB = cfg.D, cfg.C, cfg.T, cfg.L, cfg.H, cfg.KB
    J, R, RPC, KT, Q, CTU, TP = (
        cfg.J, cfg.R, cfg.RPC, cfg.KT, cfg.Q, cfg.CTU, cfg.TP,
    )
    N_CORES = cfg.n_cores
    KHF = KT // 2
    OUT_P = range(H, H + L)              # output positions 11..26

    wa_in = [
        nc.dram_tensor(f"wa{j}_in", [D, RPC], BF, kind="ExternalInput")
        for j in (1, 2, 3)
    ]
    wm_in = [
        nc.dram_tensor(f"wm{d}_in", [C, RPC], BF, kind="ExternalInput")
        for d in (0, 1, 2)
    ]
    u_in = nc.dram_tensor("u_in", [C, T], BF, kind="ExternalInput")
    const_in = nc.dram_tensor("const_in", [3, 128, RPC], F32, kind="ExternalInput")
    fx_in = nc.dram_tensor("fx_in", [2, RPC], F32, kind="ExternalInput")
    out_raw = nc.dram_tensor("out_raw", [L, J, RPC], F32, kind="ExternalOutput")

    with (
        tc.tile_pool(name="const", bufs=1) as const_pool,
        tc.tile_pool(name="state", bufs=2) as state_pool,
        tc.tile_pool(name="shard", bufs=2) as shard_pool,
        tc.tile_pool(name="outst", bufs=4) as out_pool,
        tc.tile_pool(name="psum", bufs=1, space="PSUM") as psum_pool,
        tc.tile_pool(name="dram", bufs=R - 1, space="DRAM") as dram_pool,
    ):
        # ---- resident weights / activations -------------------------------
        # u first (needed by round-0 c-GEMM), then A powers in first-use order.
        upad = const_pool.tile([128, CTU * TP], BF)
        u4 = upad[:].rearrange("p (ct t) -> p ct t", ct=CTU)
        for ct in range(CTU):
            nc.vector.memset(u4[:, ct, 0:16], 0.0)
        nc.scalar.dma_start(
            u4[:, :, 16:TP],
            u_in[:].rearrange("(ct p) t -> p ct t", p=128),
        )
        wm = []
        for d in range(3):
            t_ = const_pool.tile([128, CTU * RPC], BF, tag=f"wm{d}", name=f"wm{d}")
            nc.scalar.dma_start(
                t_[:].rearrange("p (ct m) -> p ct m", ct=CTU),
                wm_in[d][:].rearrange("(ct p) m -> p ct m", p=128),
            )
            wm.append(t_)
        cst = const_pool.tile([128, 3 * RPC], F32)
        nc.scalar.dma_start(
            cst[:].rearrange("p (j m) -> p j m", j=3),
            const_in[:].rearrange("j p m -> p j m"),
        )
        fxs = const_pool.tile([1, RPC], F32, tag="fxs", name="fxs")
        nc.scalar.dma_start(fxs[:], fx_in[0:1, :])
        fxo = const_pool.tile([1, RPC], F32, tag="fxo", name="fxo")
        nc.scalar.dma_start(fxo[:], fx_in[1:2, :])
        ident = const_pool.tile([128, 128], BF)
        make_identity(nc, ident[:])
        wa = {}
        for j in (3, 1, 2):
            t_ = const_pool.tile([128, KT * RPC], BF, tag=f"wa{j}", name=f"wa{j}")
            nc.scalar.dma_start(
                t_[:].rearrange("p (kt m) -> p kt m", kt=KT),
                wa_in[j - 1][:].rearrange("(kt p) m -> p kt m", p=128),
            )
            wa[j] = t_

        u5 = upad[:].rearrange("p (ct j l) -> p ct j l", ct=CTU, l=16)
        JB = TP // 16
        RSTAR = (H - 2) // KB            # round whose v-window covers x_0

        HRP = RPC // 2

        def fslice(h):
            # f-range of output rows: h=None -> full, else 256-row half
            return slice(0, RPC) if h is None else slice(h * HRP, (h + 1) * HRP)

        def c_gemm(ps, r, i, close, h=None):
            """Accumulate the v-correction GEMMs for position p=KB*r+i into ps."""
            fs = fslice(h)
            n = i * CTU
            m = 0
            for ii in range(1, i + 1):
                d = i - ii
                base = KB * r + ii + 16 - H
                j0, l = divmod(base, 16)
                assert 0 <= j0 and j0 + J <= JB
                for ct in range(CTU):
                    m += 1
                    nc.tensor.matmul(
                        ps[:, fs],
                        u5[:, ct, j0 : j0 + J, l],
                        wm[d][:, ct * RPC + fs.start : ct * RPC + fs.stop],
                        start=(m == 1),
                        stop=(close and m == n),
                    )

        def a_parts(xt):
            # contraction chunks ordered so the first AllGather half's rows
            # (kt = 0,1 mod 4) are consumed first
            for src in (0, 1):
                for m in range(KHF):
                    c, q2 = divmod(m, 2)
                    yield xt[src][:, m * J : (m + 1) * J], c * 4 + src * 2 + q2

        def a_main(ps, xt, i, h=None):
            """Accumulate (A^i x)^T into ps (stationary = state, moving = A^i)."""
            fs = fslice(h)
            n = 0
            for stat, kt in a_parts(xt):
                n += 1
                nc.tensor.matmul(
                    ps[:, fs],
                    stat,
                    wa[i][:, kt * RPC + fs.start : kt * RPC + fs.stop],
                    start=False,
                    stop=(n == KT),
                )

        def epilogue(ps, r, i, dst, h=None):
            """dst = ps + const_i over the f-half, with lane-0 fixups."""
            fs = fslice(h)
            cs = slice((i - 1) * RPC + fs.start, (i - 1) * RPC + fs.stop)
            nc.vector.tensor_add(dst[:, fs], ps[:, fs], cst[:, cs])
            p = KB * r + i
            if p < H - 1:
                # lane-0 halo: no bias (state is exactly zero there)
                nc.vector.tensor_copy(dst[0:1, fs], ps[0:1, fs])
            elif r == RSTAR and i == KB:
                nc.vector.tensor_add(dst[0:1, fs], ps[0:1, fs], fxs[0:1, fs])
            elif r == RSTAR and i == 2:
                nc.vector.tensor_add(dst[0:1, fs], ps[0:1, fs], fxo[0:1, fs])

        x_tiles = None
        for r in range(R):
            p3 = KB * r + KB
            # ---- state main: x_{p3} = A^3 x + c3, split into f-halves ------
            # each half transposes, exchanges, and redistributes on its own,
            # so the two 512KB AllGathers pipeline within the round
            if r < R - 1:
                ps3 = psum_pool.tile([128, RPC], F32, tag=f"ps3{r % 2}", name=f"ps3_r{r}")
                sh = shard_pool.tile([128, RPC], BF, tag="sh", name=f"sh_r{r}")
                pt = psum_pool.tile([128, RPC], BF, tag=f"pt{r % 2}", name=f"pt_r{r}")
                xsh = shard_pool.tile([128, RPC], BF, tag="xsh", name=f"xsh_r{r}")
                out_bs = []
                for h in (0, 1):
                    c_gemm(ps3, r, KB, close=(r == 0), h=h)
                    if r > 0:
                        a_main(ps3, x_tiles, KB, h=h)
                    epilogue(ps3, r, KB, sh, h=h)
                    for q in (2 * h, 2 * h + 1):
                        nc.tensor.transpose(
                            pt[:, q * 128 : (q + 1) * 128],
                            sh[:, q * 128 : (q + 1) * 128],
                            ident[:],
                        )
                    hs = fslice(h)
                    nc.vector.tensor_copy(xsh[:, hs], pt[:, hs])
                    in_b = dram_pool.tile([HRP, J], BF, tag=f"inb{h}", name=f"inb{h}_r{r}")
                    out_b = dram_pool.tile(
                        [N_CORES * HRP, J], BF, tag=f"outb{h}",
                        addr_space="Shared", name=f"outb{h}_r{r}",
                    )
                    nc.sync.dma_start(
                        in_b[:].rearrange("(q p) l -> p q l", p=128),
                        xsh[:, hs].rearrange("p (q l) -> p q l", q=2),
                    )
                    nc.gpsimd.collective_compute(
                        "AllGather",
                        mybir.AluOpType.bypass,
                        replica_groups=[list(range(N_CORES))],
                        ins=[in_b.opt()],
                        outs=[out_b.opt()],
                    )
                    out_bs.append(out_b)
                nxt = r + 1
                xn = []
                for h, tg in ((0, "xa"), (1, "xb")):
                    xt_ = state_pool.tile([128, KHF * J], BF, tag=tg, name=f"{tg}_r{nxt}")
                    nc.sync.dma_start(
                        xt_[:].rearrange("p (cq l) -> p cq l", cq=KHF),
                        out_bs[h][:].rearrange("(cq p) l -> p cq l", p=128),
                    )
                    xn.append(xt_)
                # state doubles as output at positions p3 in [H, H+L)
                if p3 in OUT_P:
                    ot = out_pool.tile([128, RPC], F32, tag="ot", name=f"ot3_r{r}")
                    epilogue(ps3, r, KB, ot)
                    nc.scalar.dma_start(out_raw[p3 - H], ot[:])
            # ---- intermediate outputs: x_{p} = A^i x + c_i, i < KB --------
            for i in range(1, KB):
                p = KB * r + i
                if p not in OUT_P:
                    continue
                psi = psum_pool.tile([128, RPC], F32, tag=f"ps{i}{r % 2}", name=f"ps{i}_r{r}")
                c_gemm(psi, r, i, close=False)
                a_main(psi, x_tiles, i)
                ot = out_pool.tile([128, RPC], F32, tag="ot", name=f"ot{i}_r{r}")
                epilogue(psi, r, i, ot)
                nc.scalar.dma_start(out_raw[p - H], ot[:])
            if r < R - 1:
                x_tiles = (xn[0], xn[1])


def make_program(cfg: Cfg = CFG):
    nc = bacc.Bacc(
        "TRN2", target_bir_lowering=False, debug=False, num_devices=cfg.n_cores
    )
    with tile.TileContext(nc) as tc:
        build(tc, cfg)
    nc.compile()
    return nc


def make_in_maps(x_0, u, W_A, b_A, W_B, b_B, cfg: Cfg = CFG):
    A = np.asarray(W_A, np.float32)
    W_B = np.asarray(W_B, np.float32)
    bt = (np.asarray(b_A) + np.asarray(b_B)).astype(np.float32)
    x_0 = np.asarray(x_0, np.float32)
    A2 = A @ A
    A3 = A2 @ A
    Abt = A @ bt
    A2bt = A2 @ bt
    const1 = bt
    const2 = bt + Abt
    const3 = bt + Abt + A2bt
    xd = x_0 - bt
    fxs_full = const3 + A2 @ xd          # replaces const3 on lane 0 at r=3, i=3
    fxo_full = const2 + A @ xd           # replaces const2 on lane 0 at r=3, i=2
    u_b = np.ascontiguousarray(np.asarray(u).astype(BF16))
    in_maps = []
    for r in range(cfg.n_cores):
        rows = slice(r * cfg.RPC, (r + 1) * cfg.RPC)
        consts = np.stack([
            np.tile(const1[rows], (128, 1)),
            np.tile(const2[rows], (128, 1)),
            np.tile(const3[rows], (128, 1)),
        ]).astype(np.float32)
        fx = np.stack([fxs_full[rows], fxo_full[rows]]).astype(np.float32)
        m = {
            "u_in": u_b,
            "const_in": np.ascontiguousarray(consts),
            "fx_in": np.ascontiguousarray(fx),
        }
        for j, Aj in ((1, A), (2, A2), (3, A3)):
            m[f"wa{j}_in"] = np.ascontiguousarray(Aj[rows, :].T.astype(BF16))
        for d, Ad in ((0, None), (1, A), (2, A2)):
            Mr = W_B[rows, :] if Ad is None else Ad[rows, :] @ W_B
            m[f"wm{d}_in"] = np.ascontiguousarray(Mr.T.astype(BF16))
        in_maps.append(m)
    return in_maps


def assemble_output(results, cfg: Cfg = CFG):
    out = np.empty((cfg.T, cfg.D), np.float32)
    for r in range(cfg.n_cores):
        raw = np.asarray(results[r]["out_raw"])      # [L, J, RPC]
        out[:, r * cfg.RPC : (r + 1) * cfg.RPC] = (
            raw.transpose(1, 0, 2).reshape(cfg.T, cfg.RPC)
        )
    return out


_CACHE: dict = {}


def kernel(**inputs):
    if "nc" not in _CACHE:
        _CACHE["nc"] = make_program()
    nc = _CACHE["nc"]
    in_maps = make_in_maps(
        inputs["x_0"], inputs["u"], inputs["W_A"],
        inputs["b_A"], inputs["W_B"], inputs["b_B"],
    )
    res = bass_utils.run_bass_kernel_spmd(
        nc, in_maps, core_ids=list(range(CFG.n_cores))
    )
    return assemble_output(res.results)
